# revision 1
# baseline (speedup 1.0000x reference)
"""Trainium2 Bass kernel for nn_KSpaceTransformerGNNEncoder (8-layer graph
transformer encoder, TransformerConv(beta=True) x8 + LN + ReLU + mean-pool).

Sharding: nodes (and their incoming edges) are partitioned across 8 NeuronCores
by destination node. Each core computes Q/K/V/skip projections for its node
shard, the K/V shards are AllGathered chip-wide each layer, and per-edge
attention uses batched DMA gathers of remote source-node rows.
"""
import sys
sys.path.insert(0, "/opt/trn_rl_repo")
import numpy as np

import concourse.bacc as bacc
import concourse.bass as bass
import concourse.mybir as mybir
import concourse.tile as tile
from concourse.bass import AP
from concourse.bass_utils import run_bass_kernel_spmd

# ---- problem constants (hardcoded per spec) ----
N, E, G = 20000, 320000, 64
F_IN, H, HEADS, LAYERS, OUT = 128, 256, 8, 8, 128
HEAD_C = H // HEADS
SCALE = 1.0 / np.sqrt(HEAD_C)
C = 8                     # cores
NLR = N // C              # real local nodes per core (2500)
NB = 20                   # node blocks per core
NL = NB * 128             # padded local nodes per core (2560)
NP = C * NL               # padded global nodes (20480)
P = 128
CH_TILES = 4              # edge tiles per gather chunk (512 edges)

F32 = mybir.dt.float32
BF16 = mybir.dt.bfloat16
I16 = mybir.dt.int16

_cache = {}


# ------------------------------------------------------------------ host prep
def _prep(x, edge_index, batch, T_blk=None):
    """Per-core index/layout preparation. Returns per-core dict arrays + T_blk."""
    src = np.asarray(edge_index[0], np.int64)
    dst = np.asarray(edge_index[1], np.int64)
    batch = np.asarray(batch, np.int64)
    x = np.asarray(x, np.float32)

    deg = np.bincount(dst, minlength=N)

    # per-core node -> (block, pos) assignment, LPT balance by in-degree
    slot_of = np.empty(N, np.int64)       # local slot within core [0, NL)
    cores = []
    for c in range(C):
        lo, hi = c * NLR, (c + 1) * NLR
        nodes = np.arange(lo, hi)
        order = nodes[np.argsort(-deg[lo:hi], kind="stable")]
        bin_load = np.zeros(NB, np.int64)
        bin_cnt = np.zeros(NB, np.int64)
        slots = np.empty(NLR, np.int64)
        for i, n in enumerate(order):
            open_bins = np.nonzero(bin_cnt < P)[0]
            b = open_bins[np.argmin(bin_load[open_bins])]
            slots[i] = b * P + bin_cnt[b]
            bin_load[b] += deg[n]
            bin_cnt[b] += 1
        slot_of[order] = slots
    pad_gid = slot_of + (np.arange(N) // NLR) * NL   # padded global id [0, NP)

    # per-core edge partition, block-sorted
    dst_core = dst // NLR
    per_core = []
    max_blk_tiles = 1
    for c in range(C):
        m = dst_core == c
        s_g, d_g = src[m], dst[m]
        d_slot = slot_of[d_g]
        blk = d_slot // P
        order = np.argsort(blk, kind="stable")
        s_g, d_slot, blk = s_g[order], d_slot[order], blk[order]
        cnt = np.bincount(blk, minlength=NB)
        max_blk_tiles = max(max_blk_tiles, int(np.ceil(cnt.max() / P)))
        per_core.append((c, s_g, d_slot, blk, cnt))

    if T_blk is None:
        T_blk = int(max_blk_tiles)
    assert max_blk_tiles <= T_blk
    EPB = T_blk * P

    EPC = CH_TILES * P

    def wrap_idx(vals):
        """vals: per-chunk flat [n_chunks, EPC] -> wrapped [128, n_chunks*EPC/16]."""
        nch = vals.shape[0]
        w = np.zeros((16, nch, EPC // 16), np.int16)
        for j in range(EPC):
            w[j % 16, :, j // 16] = vals[:, j]
        w = w.reshape(16, nch * (EPC // 16))
        return np.tile(w, (8, 1))

    out = []
    for c, s_g, d_slot, blk, cnt in per_core:
        src_pad = np.zeros(NB * EPB, np.int64)
        dstq = np.zeros(NB * EPB, np.int64)
        dstl = np.full((P, NB * T_blk), 999.0, np.float32)
        pos = 0
        for b in range(NB):
            e = int(cnt[b])
            sl = slice(b * EPB, b * EPB + e)
            src_pad[sl] = pad_gid[s_g[pos:pos + e]]
            dstq[sl] = d_slot[pos:pos + e]
            dl = (d_slot[pos:pos + e] % P).astype(np.float32)
            for i in range(e):
                t = b * T_blk + i // P
                dstl[i % P, t] = dl[i]
            pos += e
        n_chunks = NB * EPB // EPC
        src_w = wrap_idx(src_pad.reshape(n_chunks, EPC).astype(np.int16))
        dst_w = wrap_idx(dstq.reshape(n_chunks, EPC).astype(np.int16))

        batch_f = np.full((P, NB), 999.0, np.float32)
        xT = np.zeros((F_IN, NL), np.float32)
        lo = c * NLR
        loc_nodes = np.arange(lo, lo + NLR)
        loc_slots = slot_of[loc_nodes]
        batch_f[loc_slots % P, loc_slots // P] = batch[loc_nodes].astype(np.float32)
        xT[:, loc_slots] = x[loc_nodes].T
        out.append(dict(src_w=src_w, dst_w=dst_w, dstl=dstl, batch_f=batch_f, xT=xT))
    return out, T_blk


# ------------------------------------------------------------------ device build
def _build(T_blk):
    EPB = T_blk * P
    NCH_B = (T_blk + CH_TILES - 1) // CH_TILES   # gather chunks per block
    IDXW = NB * EPB // 16

    nc = bacc.Bacc("TRN2", target_bir_lowering=False, debug=False,
                   enable_asserts=True, num_devices=C)

    # ---- external inputs
    xT_d = nc.dram_tensor("xT", [F_IN, NL], F32, kind="ExternalInput")
    srcw_d = nc.dram_tensor("srcw", [P, IDXW], I16, kind="ExternalInput")
    dstw_d = nc.dram_tensor("dstw", [P, IDXW], I16, kind="ExternalInput")
    dstl_d = nc.dram_tensor("dstl", [P, NB * T_blk], F32, kind="ExternalInput")
    batch_d = nc.dram_tensor("batchf", [P, NB], F32, kind="ExternalInput")
    iota128_d = nc.dram_tensor("iota128", [P, P], F32, kind="ExternalInput")
    iota64_d = nc.dram_tensor("iota64", [P, G], F32, kind="ExternalInput")
    ident_d = nc.dram_tensor("ident", [P, P], F32, kind="ExternalInput")
    ones_d = nc.dram_tensor("ones", [P, 1], F32, kind="ExternalInput")
    winit_d = nc.dram_tensor("winit", [F_IN, H], F32, kind="ExternalInput")
    binit_d = nc.dram_tensor("binit", [P, H], F32, kind="ExternalInput")
    wq_d = nc.dram_tensor("wq", [LAYERS, H, H], F32, kind="ExternalInput")
    wk_d = nc.dram_tensor("wk", [LAYERS, H, H], F32, kind="ExternalInput")
    wv_d = nc.dram_tensor("wv", [LAYERS, H, H], F32, kind="ExternalInput")
    ws_d = nc.dram_tensor("ws", [LAYERS, H, H], F32, kind="ExternalInput")
    bq_d = nc.dram_tensor("bq", [LAYERS, P, H], F32, kind="ExternalInput")
    bk_d = nc.dram_tensor("bk", [LAYERS, P, H], F32, kind="ExternalInput")
    bv_d = nc.dram_tensor("bv", [LAYERS, P, H], F32, kind="ExternalInput")
    bs_d = nc.dram_tensor("bs", [LAYERS, P, H], F32, kind="ExternalInput")
    u_d = nc.dram_tensor("u", [LAYERS, P, H], F32, kind="ExternalInput")
    w_d = nc.dram_tensor("w", [LAYERS, P, H], F32, kind="ExternalInput")
    g_d = nc.dram_tensor("g", [LAYERS, P, H], F32, kind="ExternalInput")
    lb_d = nc.dram_tensor("lb", [LAYERS, P, H], F32, kind="ExternalInput")
    wfin_d = nc.dram_tensor("wfin", [H, OUT], F32, kind="ExternalInput")
    bfin_d = nc.dram_tensor("bfin", [P, OUT], F32, kind="ExternalInput")

    out_d = nc.dram_tensor("out", [G, OUT], F32, kind="ExternalOutput")

    # ---- internal dram
    q_dram = nc.dram_tensor("q_dram", [NL, H], F32, kind="Internal")
    kv_local = nc.dram_tensor("kv_local", [NL, 2 * H], F32, kind="Internal")
    kv_full = [nc.dram_tensor(f"kv_full{i}", [NP, 2 * H], F32, kind="Internal",
                              addr_space="Shared") for i in range(2)]
    pr_in = nc.dram_tensor("pr_in", [G, H + 1], F32, kind="Internal")
    pr_out = nc.dram_tensor("pr_out", [G, H + 1], F32, kind="Internal")

    def bcast32(ap2d, ncols):
        """[P, k] slice -> [P, k, 128] AP with 0-stride repeat of each col."""
        return AP(tensor=ap2d.tensor, offset=ap2d.offset,
                  ap=[ap2d.ap[0], [1, ncols], [0, P]])

    def rep32(ap2d, ngrp):
        """[P, ngrp*8] -> [P, ngrp, 8, 32] AP repeating each value 32x."""
        return AP(tensor=ap2d.tensor, offset=ap2d.offset,
                  ap=[ap2d.ap[0], [8, ngrp], [1, 8], [0, 32]])

    with tile.TileContext(nc) as tc:
        with (
            tc.tile_pool(name="res", bufs=1) as res,       # resident constants
            tc.tile_pool(name="hT", bufs=2) as hTp,        # transposed h, ping-pong
            tc.tile_pool(name="hN", bufs=1) as hNp,        # node-major h
            tc.tile_pool(name="wl", bufs=1) as wlp,
            tc.tile_pool(name="wl2", bufs=2) as wlp2,        # per-layer weights
            tc.tile_pool(name="gath", bufs=2) as gath,     # gather buffers
            tc.tile_pool(name="grp", bufs=2) as grp,       # per-group scratch
            tc.tile_pool(name="ep", bufs=2) as ep,         # epilogue scratch
            tc.tile_pool(name="ps", bufs=2, space="PSUM") as ps,
            tc.tile_pool(name="psb", bufs=2, space="PSUM") as psb,
            tc.tile_pool(name="pst", bufs=2, space="PSUM") as pst,
        ):
            # ---------- residents
            srcw_sb = res.tile([P, IDXW], I16)
            dstw_sb = res.tile([P, IDXW], I16)
            dstl_sb = res.tile([P, NB * T_blk], F32)
            batch_sb = res.tile([P, NB], F32)
            iota128_sb = res.tile([P, P], F32)
            iota64_sb = res.tile([P, G], F32)
            ident_sb = res.tile([P, P], F32)
            ones_sb = res.tile([P, 1], F32)
            eps_sb = res.tile([P, 1], F32)
            nc.vector.memset(eps_sb[:], 1e-5)
            for t, d in [(srcw_sb, srcw_d), (dstw_sb, dstw_d), (dstl_sb, dstl_d),
                         (batch_sb, batch_d), (iota128_sb, iota128_d),
                         (iota64_sb, iota64_d), (ident_sb, ident_d),
                         (ones_sb, ones_d)]:
                nc.sync.dma_start(t[:], d[:, :])

            # ---------- layer 0: h0 = x @ W_init + b_init (then transpose)
            xT_sb = res.tile([F_IN, NL], F32)
            nc.sync.dma_start(xT_sb[:], xT_d[:, :])
            winit_sb = res.tile([F_IN, H], F32)
            binit_sb = res.tile([P, H], F32)
            nc.sync.dma_start(winit_sb[:], winit_d[:, :])
            nc.sync.dma_start(binit_sb[:], binit_d[:, :])

            hT_cur = hTp.tile([P, 2, NL], F32, tag="hT")
            for b in range(NB):
                p0 = ps.tile([P, H], F32, space="PSUM", tag="pnode")
                nc.tensor.matmul(p0[:], xT_sb[:, bass.ts(b, P)], winit_sb[:],
                                 start=True, stop=True)
                h0 = ep.tile([P, H], F32, tag="h0")
                nc.vector.tensor_add(h0[:], p0[:], binit_sb[:])
                for kb in range(2):
                    tp = pst.tile([P, P], F32, space="PSUM", tag="ptr")
                    nc.tensor.transpose(tp[:], h0[:, bass.ts(kb, P)], ident_sb[:])
                    nc.vector.tensor_copy(hT_cur[:, kb, bass.ts(b, P)], tp[:])

            h_last = None
            for l in range(LAYERS):
                # ---------- per-layer weights into SBUF
                wq_sb = wlp.tile([P, 2, H], F32, tag="wq")
                wk_sb = wlp.tile([P, 2, H], F32, tag="wk")
                wv_sb = wlp.tile([P, 2, H], F32, tag="wv")
                ws_sb = wlp2.tile([P, 2, H], F32, tag="ws")
                for t, d in [(wq_sb, wq_d), (wk_sb, wk_d), (wv_sb, wv_d),
                             (ws_sb, ws_d)]:
                    nc.sync.dma_start(
                        t[:], d[l].rearrange("(a p) c -> p a c", p=P))
                bq_sb = wlp.tile([P, H], F32, tag="bq")
                bk_sb = wlp.tile([P, H], F32, tag="bk")
                bv_sb = wlp.tile([P, H], F32, tag="bv")
                bs_sb = wlp2.tile([P, H], F32, tag="bs")
                u_sb = wlp2.tile([P, H], F32, tag="u")
                w_sb = wlp2.tile([P, H], F32, tag="w")
                g_sb = wlp2.tile([P, H], F32, tag="g")
                lb_sb = wlp2.tile([P, H], F32, tag="lb")
                for t, d in [(bq_sb, bq_d), (bk_sb, bk_d), (bv_sb, bv_d),
                             (bs_sb, bs_d), (u_sb, u_d), (w_sb, w_d),
                             (g_sb, g_d), (lb_sb, lb_d)]:
                    nc.sync.dma_start(t[:], d[l])

                # ---------- node phase: Q, K, V per block -> DRAM
                for b in range(NB):
                    qst = ep.tile([P, H], F32, tag="qst")
                    kvst = ep.tile([P, 2 * H], F32, tag="kvst")
                    for w_t, b_t, dsti in ((wq_sb, bq_sb, 0), (wk_sb, bk_sb, 1),
                                           (wv_sb, bv_sb, 2)):
                        pp = ps.tile([P, H], F32, space="PSUM", tag="pnode")
                        nc.tensor.matmul(pp[:], hT_cur[:, 0, bass.ts(b, P)],
                                         w_t[:, 0, :], start=True, stop=False)
                        nc.tensor.matmul(pp[:], hT_cur[:, 1, bass.ts(b, P)],
                                         w_t[:, 1, :], start=False, stop=True)
                        dst_ap = (qst[:] if dsti == 0 else
                                  kvst[:, bass.ts(dsti - 1, H)])
                        nc.vector.tensor_add(dst_ap, pp[:], b_t[:])
                    nc.sync.dma_start(q_dram[bass.ts(b, P), :], qst[:])
                    nc.sync.dma_start(kv_local[bass.ts(b, P), :], kvst[:])

                # ---------- AllGather K,V
                kvf = kv_full[l % 2]
                nc.gpsimd.collective_compute(
                    "AllGather", mybir.AluOpType.bypass,
                    replica_groups=[list(range(C))],
                    ins=[kv_local[:, :]], outs=[kvf[:, :]])

                # ---------- edge phase per block
                for b in range(NB):
                    pv = psb.tile([P, H + 8], F32, space="PSUM", tag="pblk")
                    mm_i = 0
                    for ch in range(NCH_B):
                        t0 = ch * CH_TILES              # first tile in chunk
                        nt = min(CH_TILES, T_blk - t0)  # tiles this chunk
                        nidx = nt * P
                        c0 = (b * EPB + t0 * P) // 16   # idx col offset
                        kbuf = gath.tile([P, CH_TILES, H], F32, tag="kbuf")
                        vbuf = gath.tile([P, CH_TILES, H], F32, tag="vbuf")
                        qbuf = gath.tile([P, CH_TILES, H], F32, tag="qbuf")
                        nc.gpsimd.dma_gather(
                            kbuf[:, :nt, :], kvf[:, 0:H],
                            srcw_sb[:, c0:c0 + nidx // 16], nidx, nidx, H,
                            elem_step=2 * H)
                        nc.gpsimd.dma_gather(
                            vbuf[:, :nt, :], kvf[:, H:2 * H],
                            srcw_sb[:, c0:c0 + nidx // 16], nidx, nidx, H,
                            elem_step=2 * H)
                        nc.gpsimd.dma_gather(
                            qbuf[:, :nt, :], q_dram[:, :],
                            dstw_sb[:, c0:c0 + nidx // 16], nidx, nidx, H)
                        wbuf = gath.tile([P, CH_TILES, H + 8], BF16, tag="wbuf")
                        for gi in range(0, nt, 4):
                            ng = min(4, nt - gi)
                            gt = b * T_blk + t0 + gi    # global tile index
                            qk = grp.tile([P, 4 * H], F32, tag="qk")
                            nc.vector.tensor_mul(
                                qk[:, :ng * H],
                                qbuf[:, gi:gi + ng, :].rearrange("p a b -> p (a b)"),
                                kbuf[:, gi:gi + ng, :].rearrange("p a b -> p (a b)"))
                            alpha = grp.tile([P, 4 * HEADS], F32, tag="alpha")
                            nc.vector.reduce_sum(
                                alpha[:, :ng * HEADS].rearrange(
                                    "p (a h) -> p a h", h=HEADS),
                                qk[:, :ng * H].rearrange(
                                    "p (a h c) -> p a h c", h=HEADS, c=HEAD_C),
                                axis=mybir.AxisListType.X)
                            exb = grp.tile([P, 4 * H], F32, tag="exb")
                            nc.scalar.activation(
                                exb[:, :ng * H].rearrange(
                                    "p (a h c) -> p a h c", h=HEADS, c=HEAD_C),
                                rep32(alpha[:, :ng * HEADS], ng),
                                mybir.ActivationFunctionType.Exp, scale=SCALE)
                            # ex -> wbuf[., t, 256:264]
                            nc.scalar.activation(
                                wbuf[:, gi:gi + ng, H:H + 8],
                                alpha[:, :ng * HEADS].rearrange(
                                    "p (a h) -> p a h", h=HEADS),
                                mybir.ActivationFunctionType.Exp, scale=SCALE)
                            # vW -> wbuf[., t, 0:256]
                            nc.vector.tensor_mul(
                                wbuf[:, gi:gi + ng, 0:H],
                                vbuf[:, gi:gi + ng, :],
                                exb[:, :ng * H].rearrange(
                                    "p (a b) -> p a b", b=H))
                            m4 = grp.tile([P, 4, P], BF16, tag="m4")
                            nc.vector.tensor_tensor(
                                m4[:, :ng, :],
                                bcast32(dstl_sb[:, gt:gt + ng], ng),
                                AP(tensor=iota128_sb[:].tensor,
                                   offset=iota128_sb[:].offset,
                                   ap=[iota128_sb[:].ap[0], [0, ng], [1, P]]),
                                op=mybir.AluOpType.is_equal)
                            for i in range(ng):
                                nc.tensor.matmul(
                                    pv[:], m4[:, i, :], wbuf[:, gi + i, :],
                                    start=(mm_i == 0), stop=(mm_i == T_blk - 1),
                                    skip_group_check=True)
                                mm_i += 1

                    # ------- block epilogue
                    den = ep.tile([P, HEADS], F32, tag="den")
                    nc.vector.tensor_scalar_add(den[:], pv[:, H:H + 8], 1e-16)
                    rec = ep.tile([P, HEADS], F32, tag="rec")
                    nc.vector.reciprocal(rec[:], den[:])
                    recb = ep.tile([P, H], F32, tag="recb")
                    nc.vector.tensor_copy(
                        recb[:].rearrange("p (h c) -> p h c", h=HEADS),
                        rep32(rec[:], 1))
                    agg = ep.tile([P, H], F32, tag="agg")
                    nc.vector.tensor_mul(agg[:], pv[:, 0:H], recb[:])
                    # x_r = h @ Ws + bs
                    pp = ps.tile([P, H], F32, space="PSUM", tag="pnode")
                    nc.tensor.matmul(pp[:], hT_cur[:, 0, bass.ts(b, P)],
                                     ws_sb[:, 0, :], start=True, stop=False)
                    nc.tensor.matmul(pp[:], hT_cur[:, 1, bass.ts(b, P)],
                                     ws_sb[:, 1, :], start=False, stop=True)
                    x_r = ep.tile([P, H], F32, tag="x_r")
                    nc.vector.tensor_add(x_r[:], pp[:], bs_sb[:])
                    # beta = sigmoid(agg . u + x_r . w)
                    t1 = ep.tile([P, H], F32, tag="t1")
                    d1 = ep.tile([P, 1], F32, tag="d1")
                    d2 = ep.tile([P, 1], F32, tag="d2")
                    nc.vector.tensor_mul(t1[:], agg[:], u_sb[:])
                    nc.vector.reduce_sum(d1[:], t1[:], axis=mybir.AxisListType.X)
                    nc.vector.tensor_mul(t1[:], x_r[:], w_sb[:])
                    nc.vector.reduce_sum(d2[:], t1[:], axis=mybir.AxisListType.X)
                    dlog = ep.tile([P, 1], F32, tag="dlog")
                    nc.vector.tensor_add(dlog[:], d1[:], d2[:])
                    beta = ep.tile([P, 1], F32, tag="beta")
                    nc.scalar.activation(beta[:], dlog[:],
                                         mybir.ActivationFunctionType.Sigmoid)
                    # h = agg + beta * (x_r - agg)
                    dxa = ep.tile([P, H], F32, tag="dxa")
                    nc.vector.tensor_sub(dxa[:], x_r[:], agg[:])
                    hraw = ep.tile([P, H], F32, tag="hraw")
                    nc.vector.scalar_tensor_tensor(
                        out=hraw[:], in0=dxa[:], scalar=beta[:, :1], in1=agg[:],
                        op0=mybir.AluOpType.mult, op1=mybir.AluOpType.add)
                    # layernorm
                    s1 = ep.tile([P, 1], F32, tag="s1")
                    nc.vector.reduce_sum(s1[:], hraw[:], axis=mybir.AxisListType.X)
                    mu = ep.tile([P, 1], F32, tag="mu")
                    nc.scalar.mul(mu[:], s1[:], 1.0 / H)
                    xc = ep.tile([P, H], F32, tag="xc")
                    nc.vector.tensor_scalar_sub(xc[:], hraw[:], mu[:, :1])
                    sq = ep.tile([P, H], F32, tag="t1")
                    nc.vector.tensor_mul(sq[:], xc[:], xc[:])
                    ssum = ep.tile([P, 1], F32, tag="ssum")
                    nc.vector.reduce_sum(ssum[:], sq[:], axis=mybir.AxisListType.X)
                    sd = ep.tile([P, 1], F32, tag="sd")
                    nc.scalar.activation(sd[:], ssum[:],
                                         mybir.ActivationFunctionType.Sqrt,
                                         scale=1.0 / H, bias=eps_sb[:, :1])
                    rstd = ep.tile([P, 1], F32, tag="rstd")
                    nc.vector.reciprocal(rstd[:], sd[:])
                    hg = ep.tile([P, H], F32, tag="hg")
                    nc.vector.tensor_mul(hg[:], xc[:], g_sb[:])
                    hb2 = ep.tile([P, H], F32, tag="hb2")
                    nc.vector.scalar_tensor_tensor(
                        out=hb2[:], in0=hg[:], scalar=rstd[:, :1], in1=lb_sb[:],
                        op0=mybir.AluOpType.mult, op1=mybir.AluOpType.add)
                    if b == 0:
                        h_nm = hNp.tile([P, NB, H], F32, tag="h_nm")
                        hT_next = hTp.tile([P, 2, NL], F32, tag="hT")
                    nc.scalar.activation(h_nm[:, b, :], hb2[:],
                                         mybir.ActivationFunctionType.Relu)
                    for kb in range(2):
                        tp = pst.tile([P, P], F32, space="PSUM", tag="ptr")
                        nc.tensor.transpose(tp[:], h_nm[:, b, bass.ts(kb, P)],
                                            ident_sb[:])
                        nc.vector.tensor_copy(hT_next[:, kb, bass.ts(b, P)], tp[:])
                hT_cur = hT_next
                h_last = h_nm

            # ---------- global mean pool + final proj
            pp_sum = ps.tile([G, H], F32, space="PSUM", tag="pnode")
            pp_cnt = pst.tile([G, 8], F32, space="PSUM", tag="ptr")
            for b in range(NB):
                bmat = ep.tile([P, G], F32, tag="bmat")
                nc.vector.tensor_scalar(bmat[:], iota64_sb[:],
                                        batch_sb[:, b:b + 1], None,
                                        mybir.AluOpType.is_equal)
                nc.tensor.matmul(pp_sum[:], bmat[:], h_last[:, b, :],
                                 start=(b == 0), stop=(b == NB - 1),
                                 skip_group_check=True)
                nc.tensor.matmul(pp_cnt[:, 0:1], bmat[:], ones_sb[:],
                                 start=(b == 0), stop=(b == NB - 1),
                                 skip_group_check=True)
            pool_sb = ep.tile([G, H + 1], F32, tag="pool_sb")
            nc.vector.tensor_copy(pool_sb[:, 0:H], pp_sum[:])
            nc.vector.tensor_copy(pool_sb[:, H:H + 1], pp_cnt[:, 0:1])
            nc.gpsimd.dma_start(pr_in[:, :], pool_sb[:])
            nc.gpsimd.collective_compute(
                "AllReduce", mybir.AluOpType.add,
                replica_groups=[list(range(C))],
                ins=[pr_in[:, :]], outs=[pr_out[:, :]])
            red_sb = ep.tile([G, H + 1], F32, tag="red_sb")
            nc.sync.dma_start(red_sb[:], pr_out[:, :])
            cnt = ep.tile([G, 1], F32, tag="cnt")
            nc.vector.tensor_scalar_max(cnt[:], red_sb[:, H:H + 1], 1.0)
            cinv = ep.tile([G, 1], F32, tag="cinv")
            nc.vector.reciprocal(cinv[:], cnt[:])
            pooled = ep.tile([G, H], F32, tag="pooled")
            nc.vector.tensor_scalar_mul(pooled[:], red_sb[:, 0:H], cinv[:, :1])
            poolT = ep.tile([P, 2, G], F32, tag="poolT")
            for kb in range(2):
                tp = pst.tile([P, G], F32, space="PSUM", tag="ptr")
                nc.tensor.transpose(tp[:], pooled[:, bass.ts(kb, P)],
                                    ident_sb[0:G, 0:G])
                nc.vector.tensor_copy(poolT[:, kb, :], tp[:])
            wfin_sb = ep.tile([P, 2, OUT], F32, tag="wfin_sb")
            nc.sync.dma_start(wfin_sb[:],
                              wfin_d[:, :].rearrange("(a p) c -> p a c", p=P))
            bfin_sb = ep.tile([P, OUT], F32, tag="bfin_sb")
            nc.sync.dma_start(bfin_sb[:], bfin_d[:, :])
            pf = ps.tile([G, OUT], F32, space="PSUM", tag="pnode")
            nc.tensor.matmul(pf[:], poolT[:, 0, :], wfin_sb[:, 0, :],
                             start=True, stop=False)
            nc.tensor.matmul(pf[:], poolT[:, 1, :], wfin_sb[:, 1, :],
                             start=False, stop=True)
            fin = ep.tile([G, OUT], F32, tag="fin")
            nc.vector.tensor_add(fin[:], pf[:], bfin_sb[0:G, :])
            nc.sync.dma_start(out_d[:, :], fin[:])

    nc.compile()
    return nc


# ------------------------------------------------------------------ entry point
def kernel(x, edge_index, batch, W_init, b_init, Wq, bq, Wk, bk, Wv, bv,
           Ws, bs, Wbeta, ln_g, ln_b, W_final, b_final, _trace=False):
    per_core, T_blk = _prep(x, edge_index, batch)
    if T_blk not in _cache:
        _cache[T_blk] = _build(T_blk)
    nc = _cache[T_blk]

    rep = lambda v: np.tile(np.asarray(v, np.float32)[None, :], (P, 1))
    Wbeta = np.asarray(Wbeta, np.float32)
    u = Wbeta[:, 0:H, 0] + Wbeta[:, 2 * H:3 * H, 0]
    w = Wbeta[:, H:2 * H, 0] - Wbeta[:, 2 * H:3 * H, 0]
    shared = dict(
        iota128=np.tile(np.arange(P, dtype=np.float32)[None, :], (P, 1)),
        iota64=np.tile(np.arange(G, dtype=np.float32)[None, :], (P, 1)),
        ident=np.eye(P, dtype=np.float32),
        ones=np.ones((P, 1), np.float32),
        winit=np.asarray(W_init, np.float32),
        binit=rep(b_init),
        wq=np.asarray(Wq, np.float32), wk=np.asarray(Wk, np.float32),
        wv=np.asarray(Wv, np.float32), ws=np.asarray(Ws, np.float32),
        bq=np.stack([rep(bq[l]) for l in range(LAYERS)]),
        bk=np.stack([rep(bk[l]) for l in range(LAYERS)]),
        bv=np.stack([rep(bv[l]) for l in range(LAYERS)]),
        bs=np.stack([rep(bs[l]) for l in range(LAYERS)]),
        u=np.stack([rep(u[l]) for l in range(LAYERS)]),
        w=np.stack([rep(w[l]) for l in range(LAYERS)]),
        g=np.stack([rep(ln_g[l]) for l in range(LAYERS)]),
        lb=np.stack([rep(ln_b[l]) for l in range(LAYERS)]),
        wfin=np.asarray(W_final, np.float32),
        bfin=rep(b_final),
    )
    in_maps = []
    for c in range(C):
        pc = per_core[c]
        in_maps.append(dict(shared, xT=pc["xT"], srcw=pc["src_w"],
                            dstw=pc["dst_w"], dstl=pc["dstl"],
                            batchf=pc["batch_f"]))
    res = run_bass_kernel_spmd(nc, in_maps, core_ids=list(range(C)),
                               trace=_trace)
    out = res.results[0]["out"]
    if _trace:
        kernel._last_exec_ns = res.exec_time_ns
    return out


if __name__ == "__main__":
    pass



# revision 11
# speedup vs baseline: 1.6520x; 1.6520x over previous
"""Trainium2 Bass kernel for nn_KSpaceTransformerGNNEncoder (8-layer graph
transformer encoder, TransformerConv(beta=True) x8 + LN + ReLU + mean-pool).

Sharding: nodes (and their incoming edges) are partitioned across 8 NeuronCores
by destination node. Each core computes Q/K/V/skip projections for its node
shard in bf16; the K|V shard is AllGathered chip-wide each layer (bf16, in two
halves fired early so the collective overlaps the previous layer's edge
phase), and per-edge attention uses one batched bf16 DMA gather per
destination block for K|V and one for Q.
"""
import sys
sys.path.insert(0, "/opt/trn_rl_repo")
import numpy as np

import concourse.bacc as bacc
import concourse.bass as bass
import concourse.mybir as mybir
import concourse.tile as tile
from concourse.bass import AP
from concourse.bass_utils import run_bass_kernel_spmd

# ---- problem constants (hardcoded per spec) ----
N, E, G = 20000, 320000, 64
F_IN, H, HEADS, LAYERS, OUT = 128, 256, 8, 8, 128
HEAD_C = H // HEADS
SCALE = 1.0 / np.sqrt(HEAD_C)
C = 8                     # cores
NLR = N // C              # real local nodes per core (2500)
NB = 20                   # node blocks per core
NL = NB * 128             # padded local nodes per core (2560)
NP = C * NL               # padded global nodes (20480)
P = 128
SEG = NL // 2             # AllGather half size in rows (1280)

F32 = mybir.dt.float32
BF16 = mybir.dt.bfloat16
I16 = mybir.dt.int16
NPBF16 = mybir.dt.np(BF16)

_cache = {}


# ------------------------------------------------------------------ host prep
def _prep(x, edge_index, batch, T_blk=None):
    """Per-core index/layout preparation. Returns per-core dict arrays + T_blk."""
    src = np.asarray(edge_index[0], np.int64)
    dst = np.asarray(edge_index[1], np.int64)
    batch = np.asarray(batch, np.int64)
    x = np.asarray(x, np.float32)

    deg = np.bincount(dst, minlength=N)

    # per-core node -> (block, pos) assignment, LPT balance by in-degree
    slot_of = np.empty(N, np.int64)       # local slot within core [0, NL)
    for c in range(C):
        lo, hi = c * NLR, (c + 1) * NLR
        nodes = np.arange(lo, hi)
        order = nodes[np.argsort(-deg[lo:hi], kind="stable")]
        bin_load = np.zeros(NB, np.int64)
        bin_cnt = np.zeros(NB, np.int64)
        slots = np.empty(NLR, np.int64)
        for i, n in enumerate(order):
            open_bins = np.nonzero(bin_cnt < P)[0]
            b = open_bins[np.argmin(bin_load[open_bins])]
            slots[i] = b * P + bin_cnt[b]
            bin_load[b] += deg[n]
            bin_cnt[b] += 1
        slot_of[order] = slots
    # padded global row in the AllGather output layout (rank-major)
    core_of = np.arange(N) // NLR
    pad_gid = core_of * NL + slot_of

    # per-core edge partition, block-sorted
    dst_core = dst // NLR
    per_core = []
    max_blk_tiles = 1
    for c in range(C):
        m = dst_core == c
        s_g, d_g = src[m], dst[m]
        d_slot = slot_of[d_g]
        blk = d_slot // P
        order = np.argsort(blk, kind="stable")
        s_g, d_slot, blk = s_g[order], d_slot[order], blk[order]
        cnt = np.bincount(blk, minlength=NB)
        max_blk_tiles = max(max_blk_tiles, int(np.ceil(cnt.max() / P)))
        per_core.append((c, s_g, d_slot, blk, cnt))

    if T_blk is None:
        T_blk = int(max_blk_tiles)
    assert max_blk_tiles <= T_blk
    EPB = T_blk * P

    def wrap_idx(vals):
        """vals: per-block flat [NB, EPB] -> wrapped [128, NB*EPB/16]."""
        nch = vals.shape[0]
        w = np.zeros((16, nch, EPB // 16), np.int16)
        for j in range(EPB):
            w[j % 16, :, j // 16] = vals[:, j]
        w = w.reshape(16, nch * (EPB // 16))
        return np.tile(w, (8, 1))

    out = []
    for c, s_g, d_slot, blk, cnt in per_core:
        src_pad = np.zeros(NB * EPB, np.int64)
        dstq = np.zeros(NB * EPB, np.int64)
        dstl = np.full((P, NB * T_blk), 999.0, NPBF16)
        pos = 0
        for b in range(NB):
            e = int(cnt[b])
            sl = slice(b * EPB, b * EPB + e)
            src_pad[sl] = pad_gid[s_g[pos:pos + e]]
            dstq[sl] = d_slot[pos:pos + e]
            dl = (d_slot[pos:pos + e] % P).astype(np.float32)
            for i in range(e):
                t = b * T_blk + i // P
                dstl[i % P, t] = dl[i]
            pos += e
        src_w = wrap_idx(src_pad.reshape(NB, EPB).astype(np.int16))
        dst_w = wrap_idx(dstq.reshape(NB, EPB).astype(np.int16))

        batch_f = np.full((P, NB), 999.0, np.float32)
        xT = np.zeros((F_IN, NL), NPBF16)
        lo = c * NLR
        loc_nodes = np.arange(lo, lo + NLR)
        loc_slots = slot_of[loc_nodes]
        batch_f[loc_slots % P, loc_slots // P] = batch[loc_nodes].astype(np.float32)
        xT[:, loc_slots] = x[loc_nodes].astype(NPBF16).T
        out.append(dict(src_w=src_w, dst_w=dst_w, dstl=dstl, batch_f=batch_f, xT=xT))
    return out, T_blk


# ------------------------------------------------------------------ device build
def _build(T_blk):
    EPB = T_blk * P
    IDXC = EPB // 16                             # idx cols per block
    NG = (T_blk + 3) // 4                        # 4-tile groups per block

    nc = bacc.Bacc("TRN2", target_bir_lowering=False, debug=False,
                   enable_asserts=True, num_devices=C)

    # ---- external inputs
    xT_d = nc.dram_tensor("xT", [F_IN, NL], BF16, kind="ExternalInput")
    srcw_d = nc.dram_tensor("srcw", [P, NB * IDXC], I16, kind="ExternalInput")
    dstw_d = nc.dram_tensor("dstw", [P, NB * IDXC], I16, kind="ExternalInput")
    dstl_d = nc.dram_tensor("dstl", [P, NB * T_blk], BF16, kind="ExternalInput")
    batch_d = nc.dram_tensor("batchf", [P, NB], F32, kind="ExternalInput")
    iota128_d = nc.dram_tensor("iota128", [P, P], BF16, kind="ExternalInput")
    iota64_d = nc.dram_tensor("iota64", [P, G], F32, kind="ExternalInput")
    ident_d = nc.dram_tensor("ident", [P, P], BF16, kind="ExternalInput")
    identf_d = nc.dram_tensor("identf", [P, P], F32, kind="ExternalInput")
    ones_d = nc.dram_tensor("ones", [P, 1], BF16, kind="ExternalInput")
    winit_d = nc.dram_tensor("winit", [F_IN, H], BF16, kind="ExternalInput")
    binit_d = nc.dram_tensor("binit", [P, H], F32, kind="ExternalInput")
    wq_d = nc.dram_tensor("wq", [LAYERS, H, H], BF16, kind="ExternalInput")
    wk_d = nc.dram_tensor("wk", [LAYERS, H, H], BF16, kind="ExternalInput")
    wv_d = nc.dram_tensor("wv", [LAYERS, H, H], BF16, kind="ExternalInput")
    ws_d = nc.dram_tensor("ws", [LAYERS, H, H], BF16, kind="ExternalInput")
    bq_d = nc.dram_tensor("bq", [LAYERS, P, H], F32, kind="ExternalInput")
    bkv_d = nc.dram_tensor("bkv", [LAYERS, P, 2 * H], F32, kind="ExternalInput")
    bs_d = nc.dram_tensor("bs", [LAYERS, P, H], F32, kind="ExternalInput")
    u_d = nc.dram_tensor("u", [LAYERS, P, H], F32, kind="ExternalInput")
    w_d = nc.dram_tensor("w", [LAYERS, P, H], F32, kind="ExternalInput")
    g_d = nc.dram_tensor("g", [LAYERS, P, H], F32, kind="ExternalInput")
    lb_d = nc.dram_tensor("lb", [LAYERS, P, H], F32, kind="ExternalInput")
    wfin_d = nc.dram_tensor("wfin", [H, OUT], F32, kind="ExternalInput")
    bfin_d = nc.dram_tensor("bfin", [P, OUT], F32, kind="ExternalInput")

    out_d = nc.dram_tensor("out", [G, OUT], F32, kind="ExternalOutput")

    # ---- internal dram
    q_dram = nc.dram_tensor("q_dram", [NL, H], BF16, kind="Internal")
    kv_local = [nc.dram_tensor(f"kv_local{i}", [NL, 2 * H], BF16, kind="Internal")
                for i in range(2)]
    kv_full = [nc.dram_tensor(f"kv_full{i}", [NP, 2 * H], BF16, kind="Internal",
                              addr_space="Shared") for i in range(2)]
    pr_in = nc.dram_tensor("pr_in", [G, H + 1], F32, kind="Internal")
    pr_out = nc.dram_tensor("pr_out", [G, H + 1], F32, kind="Internal")

    def bcast32(ap2d, ncols):
        """[P, k] slice -> [P, k, 128] AP with 0-stride repeat of each col."""
        return AP(tensor=ap2d.tensor, offset=ap2d.offset,
                  ap=[ap2d.ap[0], [1, ncols], [0, P]])

    def rep32(ap2d, ngrp):
        """[P, ngrp*8] -> [P, ngrp, 8, 32] AP repeating each value 32x."""
        return AP(tensor=ap2d.tensor, offset=ap2d.offset,
                  ap=[ap2d.ap[0], [8, ngrp], [1, 8], [0, 32]])

    with tile.TileContext(nc) as tc:
        with (
            tc.tile_pool(name="res", bufs=1) as res,       # resident constants
            tc.tile_pool(name="hT", bufs=2) as hTp,        # transposed h, ping-pong
            tc.tile_pool(name="hN", bufs=1) as hNp,        # node-major h + x_r
            tc.tile_pool(name="wl", bufs=2) as wlp,        # per-layer weights
            tc.tile_pool(name="gath", bufs=2) as gath,     # gather buffers
            tc.tile_pool(name="grp", bufs=2) as grp,       # per-group scratch
            tc.tile_pool(name="ep", bufs=2) as ep,         # epilogue scratch
            tc.tile_pool(name="ps", bufs=2, space="PSUM") as ps,
            tc.tile_pool(name="ps2", bufs=2, space="PSUM") as ps2,
            tc.tile_pool(name="psb", bufs=2, space="PSUM") as psb,
            tc.tile_pool(name="pst", bufs=2, space="PSUM") as pst,
        ):
            # ---------- residents
            srcw_sb = res.tile([P, NB * IDXC], I16)
            dstw_sb = res.tile([P, NB * IDXC], I16)
            dstl_sb = res.tile([P, NB * T_blk], BF16)
            batch_sb = res.tile([P, NB], F32)
            iota128_sb = res.tile([P, P], BF16)
            iota64_sb = res.tile([P, G], F32)
            ident_sb = res.tile([P, P], BF16)
            identf_sb = res.tile([P, P], F32)
            ones_sb = res.tile([P, 1], BF16)
            eps_sb = res.tile([P, 1], F32)
            nc.vector.memset(eps_sb[:], 1e-5)
            for t, d in [(srcw_sb, srcw_d), (dstw_sb, dstw_d), (dstl_sb, dstl_d),
                         (batch_sb, batch_d), (iota128_sb, iota128_d),
                         (iota64_sb, iota64_d), (ident_sb, ident_d),
                         (identf_sb, identf_d), (ones_sb, ones_d)]:
                nc.sync.dma_start(t[:], d[:, :])

            # ---------- per-layer weight loads (set l live from section l-1)
            wsets = []

            def load_weights(l):
                wq_sb = wlp.tile([P, 2, H], BF16, tag="wq")
                wk_sb = wlp.tile([P, 2, H], BF16, tag="wk")
                wv_sb = wlp.tile([P, 2, H], BF16, tag="wv")
                ws_sb = wlp.tile([P, 2, H], BF16, tag="ws")
                for t, d in [(wq_sb, wq_d), (wk_sb, wk_d), (wv_sb, wv_d),
                             (ws_sb, ws_d)]:
                    nc.sync.dma_start(
                        t[:], d[l].rearrange("(a p) c -> p a c", p=P))
                bq_sb = wlp.tile([P, H], F32, tag="bq")
                bkv_sb = wlp.tile([P, 2 * H], F32, tag="bkv")
                bs_sb = wlp.tile([P, H], F32, tag="bs")
                u_sb = wlp.tile([P, H], F32, tag="u")
                w_sb = wlp.tile([P, H], F32, tag="w")
                g_sb = wlp.tile([P, H], F32, tag="g")
                lb_sb = wlp.tile([P, H], F32, tag="lb")
                for t, d in [(bq_sb, bq_d), (bkv_sb, bkv_d), (bs_sb, bs_d),
                             (u_sb, u_d), (w_sb, w_d), (g_sb, g_d),
                             (lb_sb, lb_d)]:
                    nc.sync.dma_start(t[:], d[l])
                wsets.append(dict(wq=wq_sb, wk=wk_sb, wv=wv_sb, ws=ws_sb,
                                  bq=bq_sb, bkv=bkv_sb, bs=bs_sb, u=u_sb,
                                  w=w_sb, g=g_sb, lb=lb_sb))

            def proj2(hT, b, w_sb, half):
                """[128 nodes, H] = hT_block @ W (2 contraction chunks)."""
                pp = ps.tile([P, H], F32, space="PSUM", tag="pnode")
                nc.tensor.matmul(pp[:], hT[:, 0, bass.ts(b, P)], w_sb[:, 0, :],
                                 start=True, stop=False, skip_group_check=True)
                nc.tensor.matmul(pp[:], hT[:, 1, bass.ts(b, P)], w_sb[:, 1, :],
                                 start=False, stop=True, skip_group_check=True)
                return pp

            def kvproj(l, hT, b):
                """K|V projection of block b -> kv_local[l % 2]."""
                ws_ = wsets[l]
                pk = proj2(hT, b, ws_["wk"], 0)
                pvv = proj2(hT, b, ws_["wv"], 1)
                kvst = ep.tile([P, 2 * H], BF16, tag="kvst")
                nc.vector.tensor_add(kvst[:, 0:H], pk[:], ws_["bkv"][:, 0:H])
                nc.vector.tensor_add(kvst[:, H:2 * H], pvv[:],
                                     ws_["bkv"][:, H:2 * H])
                nc.sync.dma_start(kv_local[l % 2][bass.ts(b, P), :], kvst[:])

            def ag_full(l):
                nc.gpsimd.collective_compute(
                    "AllGather", mybir.AluOpType.bypass,
                    replica_groups=[list(range(C))],
                    ins=[kv_local[l % 2][:, :]],
                    outs=[kv_full[l % 2][:, :]])

            def qxr(l, hT, b, x_r_sb):
                """Q projection -> q_dram; x_r -> SBUF, for block b."""
                ws_ = wsets[l]
                pq = proj2(hT, b, ws_["wq"], 0)
                qst = ep.tile([P, H], BF16, tag="qst")
                nc.vector.tensor_add(qst[:], pq[:], ws_["bq"][:])
                nc.sync.dma_start(q_dram[bass.ts(b, P), :], qst[:])
                px = proj2(hT, b, ws_["ws"], 1)
                nc.vector.tensor_add(x_r_sb[:, b, :], px[:], ws_["bs"][:])

            # ---------- prologue: h0 = x @ W_init, then KV(0), AG(0), Q(0)
            xT_sb = res.tile([F_IN, NL], BF16)
            nc.sync.dma_start(xT_sb[:], xT_d[:, :])
            winit_sb = res.tile([F_IN, H], BF16)
            binit_sb = res.tile([P, H], F32)
            nc.sync.dma_start(winit_sb[:], winit_d[:, :])
            nc.sync.dma_start(binit_sb[:], binit_d[:, :])
            load_weights(0)

            hT_cur = hTp.tile([P, 2, NL], BF16, tag="hT")
            for b in range(NB):
                p0 = ps.tile([P, H], F32, space="PSUM", tag="pnode")
                nc.tensor.matmul(p0[:], xT_sb[:, bass.ts(b, P)], winit_sb[:],
                                 start=True, stop=True, skip_group_check=True)
                h0 = ep.tile([P, H], BF16, tag="h0")
                nc.vector.tensor_add(h0[:], p0[:], binit_sb[:])
                for kb in range(2):
                    tp = pst.tile([P, P], BF16, space="PSUM", tag="ptr")
                    nc.tensor.transpose(tp[:], h0[:, bass.ts(kb, P)], ident_sb[:])
                    nc.vector.tensor_copy(hT_cur[:, kb, bass.ts(b, P)], tp[:])
            for b in range(NB):
                kvproj(0, hT_cur, b)
            ag_full(0)
            x_r_sb = hNp.tile([P, NB, H], F32, tag="x_r")
            for b in range(NB):
                qxr(0, hT_cur, b, x_r_sb)

            h_nm = None
            for l in range(LAYERS):
                if l < LAYERS - 1:
                    load_weights(l + 1)
                ws_ = wsets[l]
                kvf = kv_full[l % 2]
                hT_next = hTp.tile([P, 2, NL], BF16, tag="hT")
                h_nm = hNp.tile([P, NB, H], BF16, tag="h_nm")

                for b in range(NB):
                    # ------- gathers (KV + Q per block, <=1024 idxs per call:
                    # the SWDGE ring holds 128 descs/engine; 2048 idxs = 128
                    # exactly fills it and wedges the DMA unit)
                    kvbuf = gath.tile([P, T_blk, 2 * H], BF16, tag="kvbuf")
                    qbuf = gath.tile([P, T_blk, H], BF16, tag="qbuf")
                    CH = 1024
                    for ch in range((EPB + CH - 1) // CH):
                        nI = min(CH, EPB - ch * CH)
                        nT = nI // P
                        c0 = b * IDXC + ch * (CH // 16)
                        nc.gpsimd.dma_gather(
                            kvbuf[:, ch * (CH // P):ch * (CH // P) + nT, :],
                            kvf[:, :], srcw_sb[:, c0:c0 + nI // 16],
                            nI, nI, 2 * H)
                        nc.gpsimd.dma_gather(
                            qbuf[:, ch * (CH // P):ch * (CH // P) + nT, :],
                            q_dram[:, :], dstw_sb[:, c0:c0 + nI // 16],
                            nI, nI, H)

                    pv = psb.tile([P, H + 8], F32, space="PSUM", tag="pblk")
                    mm_i = 0
                    for gi4 in range(NG):
                        t0 = gi4 * 4
                        ng = min(4, T_blk - t0)
                        qk = grp.tile([P, 4, H], BF16, tag="qk")
                        nc.vector.tensor_mul(
                            qk[:, :ng, :],
                            qbuf[:, t0:t0 + ng, :],
                            kvbuf[:, t0:t0 + ng, 0:H])
                        alpha = grp.tile([P, 4, HEADS], F32, tag="alpha")
                        nc.vector.reduce_sum(
                            alpha[:, :ng, :],
                            qk[:, :ng, :].rearrange(
                                "p a (h c) -> p a h c", h=HEADS),
                            axis=mybir.AxisListType.X)
                        wbuf = grp.tile([P, 4, H + 8], BF16, tag="wbuf")
                        # ex -> wbuf[., t, 256:264] (denominator lanes)
                        nc.scalar.activation(
                            wbuf[:, :ng, H:H + 8], alpha[:, :ng, :],
                            mybir.ActivationFunctionType.Exp, scale=SCALE)
                        # exb = exp(alpha) broadcast 32x, then vW = v * exb
                        exb = grp.tile([P, 4, H], BF16, tag="exb")
                        nc.scalar.activation(
                            exb[:, :ng, :].rearrange(
                                "p a (h c) -> p a h c", h=HEADS),
                            rep32(alpha[:].rearrange("p a h -> p (a h)"), ng),
                            mybir.ActivationFunctionType.Exp, scale=SCALE)
                        nc.vector.tensor_mul(
                            wbuf[:, :ng, 0:H],
                            kvbuf[:, t0:t0 + ng, H:2 * H],
                            exb[:, :ng, :])
                        gt = b * T_blk + t0
                        m4 = grp.tile([P, 4, P], BF16, tag="m4")
                        nc.vector.tensor_tensor(
                            m4[:, :ng, :],
                            bcast32(dstl_sb[:, gt:gt + ng], ng),
                            AP(tensor=iota128_sb[:].tensor,
                               offset=iota128_sb[:].offset,
                               ap=[iota128_sb[:].ap[0], [0, ng], [1, P]]),
                            op=mybir.AluOpType.is_equal)
                        for i in range(ng):
                            nc.tensor.matmul(
                                pv[:], m4[:, i, :], wbuf[:, i, :],
                                start=(mm_i == 0), stop=(mm_i == T_blk - 1),
                                skip_group_check=True)
                            mm_i += 1

                    # ------- block epilogue
                    den = ep.tile([P, HEADS], F32, tag="den")
                    nc.vector.tensor_scalar_add(den[:], pv[:, H:H + 8], 1e-16)
                    rec = ep.tile([P, HEADS], F32, tag="rec")
                    nc.vector.reciprocal(rec[:], den[:])
                    agg = ep.tile([P, H], F32, tag="agg")
                    nc.vector.tensor_mul(agg[:], pv[:, 0:H], rep32(rec[:], 1))
                    # beta = sigmoid(agg . u + x_r . w)
                    tj1 = ep.tile([P, H], F32, tag="tj1")
                    tj2 = ep.tile([P, H], F32, tag="tj2")
                    d1 = ep.tile([P, 1], F32, tag="d1")
                    d2 = ep.tile([P, 1], F32, tag="d2")
                    dlog = ep.tile([P, 1], F32, tag="dlog")
                    nc.vector.tensor_mul(tj1[:], agg[:], ws_["u"][:])
                    nc.vector.reduce_sum(d1[:], tj1[:],
                                         axis=mybir.AxisListType.X)
                    nc.vector.tensor_mul(tj2[:], x_r_sb[:, b, :], ws_["w"][:])
                    nc.vector.reduce_sum(d2[:], tj2[:],
                                         axis=mybir.AxisListType.X)
                    nc.vector.tensor_add(dlog[:], d1[:], d2[:])
                    beta = ep.tile([P, 1], F32, tag="beta")
                    nc.scalar.activation(beta[:], dlog[:],
                                         mybir.ActivationFunctionType.Sigmoid)
                    # h = agg + beta * (x_r - agg)
                    dxa = ep.tile([P, H], F32, tag="dxa")
                    nc.vector.tensor_sub(dxa[:], x_r_sb[:, b, :], agg[:])
                    hraw = ep.tile([P, H], F32, tag="hraw")
                    nc.vector.scalar_tensor_tensor(
                        out=hraw[:], in0=dxa[:], scalar=beta[:, :1], in1=agg[:],
                        op0=mybir.AluOpType.mult, op1=mybir.AluOpType.add)
                    # layernorm (mean via ACT accumulate)
                    mj = ep.tile([P, H], BF16, tag="mj")
                    mu = ep.tile([P, 1], F32, tag="mu")
                    nc.scalar.activation(mj[:], hraw[:],
                                         mybir.ActivationFunctionType.Copy,
                                         scale=1.0 / H, accum_out=mu[:])
                    xc = ep.tile([P, H], F32, tag="xc")
                    nc.vector.tensor_scalar_sub(xc[:], hraw[:], mu[:, :1])
                    sq = ep.tile([P, H], F32, tag="sq")
                    ssum = ep.tile([P, 1], F32, tag="ssum")
                    nc.vector.tensor_mul(sq[:], xc[:], xc[:])
                    nc.vector.reduce_sum(ssum[:], sq[:],
                                         axis=mybir.AxisListType.X)
                    sd = ep.tile([P, 1], F32, tag="sd")
                    nc.scalar.activation(sd[:], ssum[:],
                                         mybir.ActivationFunctionType.Sqrt,
                                         scale=1.0 / H, bias=eps_sb[:, :1])
                    rstd = ep.tile([P, 1], F32, tag="rstd")
                    nc.vector.reciprocal(rstd[:], sd[:])
                    hg = ep.tile([P, H], F32, tag="hg")
                    nc.vector.tensor_mul(hg[:], xc[:], ws_["g"][:])
                    hb2 = ep.tile([P, H], F32, tag="hb2")
                    nc.vector.scalar_tensor_tensor(
                        out=hb2[:], in0=hg[:], scalar=rstd[:, :1],
                        in1=ws_["lb"][:],
                        op0=mybir.AluOpType.mult, op1=mybir.AluOpType.add)
                    nc.scalar.activation(h_nm[:, b, :], hb2[:],
                                         mybir.ActivationFunctionType.Relu)
                    for kb in range(2):
                        tp = pst.tile([P, P], BF16, space="PSUM", tag="ptr")
                        nc.tensor.transpose(tp[:], h_nm[:, b, bass.ts(kb, P)],
                                            ident_sb[:])
                        nc.vector.tensor_copy(hT_next[:, kb, bass.ts(b, P)],
                                              tp[:])
                    # interleave next layer's K|V projection
                    if l < LAYERS - 1:
                        kvproj(l + 1, hT_next, b)
                        if b == NB - 1:
                            ag_full(l + 1)

                if l < LAYERS - 1:
                    x_r_sb = hNp.tile([P, NB, H], F32, tag="x_r")
                    for b in range(NB):
                        qxr(l + 1, hT_next, b, x_r_sb)
                hT_cur = hT_next

            # ---------- global mean pool + final proj
            pp_sum = ps.tile([G, H], F32, space="PSUM", tag="pnode")
            pp_cnt = pst.tile([G, 8], F32, space="PSUM", tag="ptr")
            for b in range(NB):
                bmat = ep.tile([P, G], BF16, tag="bmat")
                nc.vector.tensor_scalar(bmat[:], iota64_sb[:],
                                        batch_sb[:, b:b + 1], None,
                                        mybir.AluOpType.is_equal)
                nc.tensor.matmul(pp_sum[:], bmat[:], h_nm[:, b, :],
                                 start=(b == 0), stop=(b == NB - 1),
                                 skip_group_check=True)
                nc.tensor.matmul(pp_cnt[:, 0:1], bmat[:], ones_sb[:],
                                 start=(b == 0), stop=(b == NB - 1),
                                 skip_group_check=True)
            pool_sb = ep.tile([G, H + 1], F32, tag="pool_sb")
            nc.vector.tensor_copy(pool_sb[:, 0:H], pp_sum[:])
            nc.vector.tensor_copy(pool_sb[:, H:H + 1], pp_cnt[:, 0:1])
            nc.gpsimd.dma_start(pr_in[:, :], pool_sb[:])
            nc.gpsimd.collective_compute(
                "AllReduce", mybir.AluOpType.add,
                replica_groups=[list(range(C))],
                ins=[pr_in[:, :]], outs=[pr_out[:, :]])
            red_sb = ep.tile([G, H + 1], F32, tag="red_sb")
            nc.sync.dma_start(red_sb[:], pr_out[:, :])
            cnt = ep.tile([G, 1], F32, tag="cnt")
            nc.vector.tensor_scalar_max(cnt[:], red_sb[:, H:H + 1], 1.0)
            cinv = ep.tile([G, 1], F32, tag="cinv")
            nc.vector.reciprocal(cinv[:], cnt[:])
            pooled = ep.tile([G, H], F32, tag="pooled")
            nc.vector.tensor_scalar_mul(pooled[:], red_sb[:, 0:H], cinv[:, :1])
            poolT = ep.tile([P, 2, G], F32, tag="poolT")
            for kb in range(2):
                tp = pst.tile([P, G], F32, space="PSUM", tag="ptr")
                nc.tensor.transpose(tp[:], pooled[:, bass.ts(kb, P)],
                                    identf_sb[0:G, 0:G])
                nc.vector.tensor_copy(poolT[:, kb, :], tp[:])
            wfin_sb = ep.tile([P, 2, OUT], F32, tag="wfin_sb")
            nc.sync.dma_start(wfin_sb[:],
                              wfin_d[:, :].rearrange("(a p) c -> p a c", p=P))
            bfin_sb = ep.tile([P, OUT], F32, tag="bfin_sb")
            nc.sync.dma_start(bfin_sb[:], bfin_d[:, :])
            pf = ps.tile([G, OUT], F32, space="PSUM", tag="pnode")
            nc.tensor.matmul(pf[:], poolT[:, 0, :], wfin_sb[:, 0, :],
                             start=True, stop=False, skip_group_check=True)
            nc.tensor.matmul(pf[:], poolT[:, 1, :], wfin_sb[:, 1, :],
                             start=False, stop=True, skip_group_check=True)
            fin = ep.tile([G, OUT], F32, tag="fin")
            nc.vector.tensor_add(fin[:], pf[:], bfin_sb[0:G, :])
            nc.sync.dma_start(out_d[:, :], fin[:])

    nc.compile()
    return nc


# ------------------------------------------------------------------ entry point
def kernel(x, edge_index, batch, W_init, b_init, Wq, bq, Wk, bk, Wv, bv,
           Ws, bs, Wbeta, ln_g, ln_b, W_final, b_final, _trace=False):
    per_core, T_blk = _prep(x, edge_index, batch)
    if T_blk not in _cache:
        _cache[T_blk] = _build(T_blk)
    nc = _cache[T_blk]

    rep = lambda v: np.tile(np.asarray(v, np.float32)[None, :], (P, 1))
    bf = lambda v: np.asarray(v, np.float32).astype(NPBF16)
    Wbeta = np.asarray(Wbeta, np.float32)
    u = Wbeta[:, 0:H, 0] + Wbeta[:, 2 * H:3 * H, 0]
    w = Wbeta[:, H:2 * H, 0] - Wbeta[:, 2 * H:3 * H, 0]
    bkv = np.concatenate([np.asarray(bk, np.float32),
                          np.asarray(bv, np.float32)], axis=1)
    shared = dict(
        iota128=np.tile(np.arange(P, dtype=np.float32)[None, :],
                        (P, 1)).astype(NPBF16),
        iota64=np.tile(np.arange(G, dtype=np.float32)[None, :], (P, 1)),
        ident=np.eye(P, dtype=np.float32).astype(NPBF16),
        identf=np.eye(P, dtype=np.float32),
        ones=np.ones((P, 1), np.float32).astype(NPBF16),
        winit=bf(W_init),
        binit=rep(b_init),
        wq=bf(Wq), wk=bf(Wk), wv=bf(Wv), ws=bf(Ws),
        bq=np.stack([rep(bq[l]) for l in range(LAYERS)]),
        bkv=np.stack([rep(bkv[l]) for l in range(LAYERS)]),
        bs=np.stack([rep(bs[l]) for l in range(LAYERS)]),
        u=np.stack([rep(u[l]) for l in range(LAYERS)]),
        w=np.stack([rep(w[l]) for l in range(LAYERS)]),
        g=np.stack([rep(ln_g[l]) for l in range(LAYERS)]),
        lb=np.stack([rep(ln_b[l]) for l in range(LAYERS)]),
        wfin=np.asarray(W_final, np.float32),
        bfin=rep(b_final),
    )
    in_maps = []
    for c in range(C):
        pc = per_core[c]
        in_maps.append(dict(shared, xT=pc["xT"], srcw=pc["src_w"],
                            dstw=pc["dst_w"], dstl=pc["dstl"],
                            batchf=pc["batch_f"]))
    res = run_bass_kernel_spmd(nc, in_maps, core_ids=list(range(C)),
                               trace=_trace)
    out = res.results[0]["out"]
    if _trace:
        kernel._last_exec_ns = res.exec_time_ns
    return out


if __name__ == "__main__":
    pass
